# revision 1
# baseline (speedup 1.0000x reference)
"""Trainium2 Bass kernel for nn_DepthMemoryCache.

Reference computation (D=8, B=4, S=4096, C=1024, G=64):
    u     = einsum('bsc,gc->bsg', x[-1], W_u)
    keys  = einsum('dbc,gc->dbg', x.mean(2), W_u)
    gates = softmax(einsum('bsg,dbg->bsd', u, keys), axis=-1)
    out   = einsum('dbsc,bsd->bsc', x, gates)

Strategy: shard the sequence axis over 8 cores (core i gets
x[:, :, i*512:(i+1)*512, :]). Per core, two streaming passes over the 64MB
shard:
  A) depth/batch sums over s on PE: slabs are cast to bf16 (on the otherwise
     idle DVE/ACT engines) and column-summed with indicator stationaries in a
     single PSUM accumulation region. For the d=D-1 slabs, uT = W_u @ x7.T is
     also computed on PE (bf16 transposes + matmuls) so phase B needs no
     per-block transposes. A 128KB all-core AllReduce completes the
     full-sequence means (a tiny warm-up AllReduce at kernel start absorbs
     comm setup under phase A; collective bounce DMAs ride GpSimd's queue so
     the Sync engine keeps issuing prefetch reads).
  B) after a short fixup (meanT transposes + keysT matmuls), each 128-row
     block needs ONE small matmul for logits, softmax via ACT exp with
     accum_out, then 8 streamed depth tiles combined by fused
     scalar_tensor_tensor FMAs (fp32, exact) with per-partition gate scalars
     on DVE; gates are interleaved with streaming so the first FMA fires
     right after the collective.
HBM traffic per core: 64 (A) + 64 (B) + 8 (write) = 136MB.
The bf16 mean/logit paths cost ~1e-3/2e-4 relative on gates only; the output
weighted sum stays fp32.
"""
import sys

sys.path.insert(0, "/opt/trn_rl_repo")

from contextlib import ExitStack

import numpy as np
from concourse import bacc, bass, mybir, tile, masks
from concourse import bass_utils

F32 = mybir.dt.float32
BF16 = mybir.dt.bfloat16

D, B, S, C, G = 8, 4, 4096, 1024, 64
N_CORES = 8
P = 128                 # partition count / block rows
NKC = C // P            # 8 column chunks of 128


def build_body(tc, x, w, y, s_sh):
    """Emit the kernel IR. x:[D,B,s_sh,C], w:[G,C], y:[B,s_sh,C] dram APs."""
    nc = tc.nc
    nj = s_sh // P      # 128-row blocks per (d, b)
    mul, add = mybir.AluOpType.mult, mybir.AluOpType.add
    DB = D * B
    es = ExitStack()

    singles = es.enter_context(tc.tile_pool(name="singles", bufs=1))
    ident = singles.tile([P, P], F32)
    masks.make_identity(nc, ident[:])
    ident_bf = singles.tile([P, P], BF16)
    masks.make_identity(nc, ident_bf[:])
    # indicator stationaries: ind[:, r, m] = (m == r) / S  — column-sums a
    # bf16 slab into psum row r with one N=512 matmul per c-half.
    ind_bf = singles.tile([P, DB, DB], BF16)
    nc.vector.memset(ind_bf[:], 0.0)
    for r in range(DB):
        nc.vector.memset(ind_bf[:, r, r:r + 1], 1.0 / (N_CORES * s_sh))
    w_sb = singles.tile([G, C], F32)
    nc.sync.dma_start(w_sb[:], w[:])
    x7bf_sb = singles.tile([P, B, nj, C], BF16)
    gates_sb = singles.tile([P, B, nj, D], F32)
    sums_sb = singles.tile([DB, C], F32)
    sumk_sb = singles.tile([G, B * D], F32)
    meanT_sb = singles.tile([P, NKC * DB], F32)
    wT_sb = singles.tile([P, NKC, G], F32)
    wT_bf = singles.tile([P, NKC, G], BF16)
    keysT_sb = singles.tile([G, B, D], F32)
    uT_sb = singles.tile([G, B, nj, P], F32)

    stream = es.enter_context(tc.tile_pool(name="stream", bufs=3))
    bfp = es.enter_context(tc.tile_pool(name="bfp", bufs=2))

    dram = es.enter_context(tc.tile_pool(name="dram", bufs=1, space="DRAM"))
    # tiny warm-up AllReduce: absorbs collective-comm setup under phase A
    ccw_in = dram.tile([1, 16], F32)
    ccw_out = dram.tile([1, 16], F32)
    cc_in = dram.tile([G, B * D], F32)
    cc_out = dram.tile([G, B * D], F32)
    warm_sb = singles.tile([1, 16], F32)
    nc.vector.memset(warm_sb[:], 0.0)
    nc.gpsimd.dma_start(ccw_in[:], warm_sb[:])
    nc.gpsimd.collective_compute(
        "AllReduce", add, replica_groups=[list(range(N_CORES))],
        ins=[ccw_in.opt()], outs=[ccw_out.opt()],
    )

    # ---------------- Phase A: partial sums over s (scaled by 1/S) ----------
    with tc.tile_pool(name="psumA", bufs=1, space="PSUM") as psA, \
         tc.tile_pool(name="psumT", bufs=1, space="PSUM") as psT, \
         tc.tile_pool(name="psumXA", bufs=3, space="PSUM") as psXA, \
         tc.tile_pool(name="psumU", bufs=2, space="PSUM") as psU, \
         tc.tile_pool(name="xtA", bufs=3) as xtA:
        sums_ps = psA.tile([DB, C], F32)

        # Each 512-col half of sums_ps is one 2KB PSUM zero region: start=True
        # zeroes the WHOLE region, so exactly one start (global first MM into
        # that region) / one stop (global last); every other matmul
        # accumulates onto pending-zero bytes. Rows m != r get +0.
        def sum_slab(slab_bf, d, b, first, last):
            r = d * B + b
            for h in range(2):
                for j in range(nj):
                    nc.tensor.matmul(
                        sums_ps[:, h * 512:(h + 1) * 512],
                        ind_bf[:, r, :],
                        slab_bf[:, j, h * 512:(h + 1) * 512],
                        start=(first and j == 0),
                        stop=(last and j == nj - 1),
                    )

        def cast_slab(dst_bf, src_f32, i):
            # split the fp32->bf16 casts between DVE and ACT (both idle here)
            for j in range(nj):
                if (i * nj + j) % 2 == 0:
                    nc.vector.tensor_copy(dst_bf[:, j, :], src_f32[:, j, :])
                else:
                    nc.scalar.copy(dst_bf[:, j, :], src_f32[:, j, :])

        # one-time W_u transpose: wT[c, g] chunks (fp32 + bf16 copies)
        for k in range(NKC):
            tr = psT.tile([P, NKC * DB], F32, tag="fix")
            nc.tensor.transpose(tr[:, :G], w_sb[:, k * P:(k + 1) * P], ident[:G, :G])
            nc.vector.tensor_copy(wT_sb[:, k, :], tr[:, :G])
            nc.scalar.copy(wT_bf[:, k, :], tr[:, :G])

        def u_block(b, j):
            # uT[g, s-block] = sum_k (wT_k).T @ x7T_k on PE (reads resident
            # x7bf, so this can run any time after the d=D-1 cast)
            u_ps = psU.tile([G, P], F32, tag="u")
            for k in range(NKC):
                xt_ps = psXA.tile([P, P], BF16, tag="xt_ps")
                nc.tensor.transpose(
                    xt_ps[:], x7bf_sb[:, b, j, k * P:(k + 1) * P],
                    ident_bf[:])
                xt_sb = xtA.tile([P, P], BF16, tag="xt_sb")
                if k % 2 == 0:
                    nc.scalar.copy(xt_sb[:], xt_ps[:])
                else:
                    nc.vector.tensor_copy(xt_sb[:], xt_ps[:])
                nc.tensor.matmul(
                    u_ps[:], wT_bf[:, k, :], xt_sb[:],
                    start=(k == 0), stop=(k == NKC - 1))
            nc.vector.tensor_copy(uT_sb[:, b, j, :], u_ps[:])

        # d = 7 first (fills the resident x7bf); one uT block is interleaved
        # after every later slab so the PE/copy work spreads over phase A
        ublocks = [(b, j) for b in range(B) for j in range(nj)]
        ub_i = 0
        for dd in range(D):
            d = (dd + D - 1) % D
            for b in range(B):
                slab = stream.tile([P, nj, C], F32, tag="slab")
                nc.sync.dma_start(
                    slab[:], x[d, b].rearrange("(j p) c -> p j c", p=P))
                if d == D - 1:
                    xbf = x7bf_sb[:, b]
                else:
                    xbf_t = bfp.tile([P, nj, C], BF16, tag="xbf")
                    xbf = xbf_t[:]
                cast_slab(xbf, slab[:], d * B + b)
                sum_slab(xbf, d, b, first=(dd == 0 and b == 0),
                         last=(dd == D - 1 and b == B - 1))
                if dd >= 1 and ub_i < len(ublocks):
                    ub, uj = ublocks[ub_i]; ub_i += 1
                    u_block(ub, uj)
        while ub_i < len(ublocks):
            ub, uj = ublocks[ub_i]; ub_i += 1
            u_block(ub, uj)

        nc.vector.tensor_copy(sums_sb[:], sums_ps[:])

        # ---- local partial keysT (keys are linear in the means, so the ----
        # ---- AllReduce can run in the tiny keys space: 8KB not 128KB)  ----
        # meanT[c, (d,b)] chunks via PE transpose — all 8 into one psum tile
        # (one zero region => single start/stop accumulation group)
        mt_ps = psT.tile([P, NKC * DB], F32, tag="fix")
        for k in range(NKC):
            nc.tensor.matmul(
                mt_ps[:, k * DB:(k + 1) * DB],
                sums_sb[:, k * P:(k + 1) * P], ident[:DB, :DB],
                is_transpose=True, start=(k == 0), stop=(k == NKC - 1))
        nc.vector.tensor_copy(meanT_sb[:], mt_ps[:])
        # partial keysT[g, d] per b = sum_k wT_k.T @ meanT_k
        keys_ps = psT.tile([P, NKC * DB], F32, tag="fix")
        for b in range(B):
            for k in range(NKC):
                nc.tensor.matmul(
                    keys_ps[:G, b * D:(b + 1) * D],
                    wT_sb[:, k, :],
                    meanT_sb[:, k * DB:(k + 1) * DB].rearrange(
                        "p (d b) -> p d b", b=B)[:, :, b],
                    start=(k == 0), stop=(k == NKC - 1),
                )
        nc.vector.tensor_copy(sumk_sb[:], keys_ps[:G, :B * D])

    # ---------------- AllReduce the [G, B*D] partial keys -------------------
    # bounce DMAs go through GpSimd's queue so the Sync engine never blocks
    # on the collective and keeps issuing phase-B prefetch reads.
    nc.gpsimd.dma_start(cc_in[:], sumk_sb[:])
    nc.gpsimd.collective_compute(
        "AllReduce", add,
        replica_groups=[list(range(N_CORES))],
        ins=[cc_in.opt()], outs=[cc_out.opt()],
    )
    nc.gpsimd.dma_start(
        keysT_sb[:].rearrange("g b d -> g (b d)"), cc_out[:])

    # ---------------- Phase B: gates + depth-weighted sum -------------------
    with tc.tile_pool(name="psumL", bufs=2, space="PSUM") as psL, \
         tc.tile_pool(name="bstream", bufs=14) as bstream, \
         tc.tile_pool(name="accp", bufs=4) as accp, \
         tc.tile_pool(name="small", bufs=4) as small:
        for b in range(B):
            for j in range(nj):
                # logits for this block: one small matmul off resident uT
                lg_ps = psL.tile([P, D], F32, tag="lg")
                nc.tensor.matmul(lg_ps[:], uT_sb[:, b, j, :], keysT_sb[:, b, :])
                e_sb = small.tile([P, D], F32, tag="e")
                z_sb = small.tile([P, 1], F32, tag="z")
                rz_sb = small.tile([P, 1], F32, tag="rz")
                nc.scalar.activation(
                    e_sb[:], lg_ps[:], mybir.ActivationFunctionType.Exp,
                    accum_out=z_sb[:])
                nc.vector.reciprocal(rz_sb[:], z_sb[:])
                nc.scalar.mul(gates_sb[:, b, j, :], e_sb[:], rz_sb[:])

                acc = accp.tile([P, C], F32, tag="acc")
                for dd in range(D):
                    d = (dd + D - 1) % D        # d = 7 first, then 0..6
                    t = bstream.tile([P, C], F32, tag="bslab")
                    nc.sync.dma_start(
                        t[:], x[d, b, j * P:(j + 1) * P, :])
                    if dd == 0:
                        nc.vector.tensor_scalar_mul(
                            acc[:], t[:], gates_sb[:, b, j, d:d + 1])
                    else:
                        nc.vector.scalar_tensor_tensor(
                            out=acc[:], in0=t[:],
                            scalar=gates_sb[:, b, j, d:d + 1],
                            in1=acc[:], op0=mul, op1=add)
                # y writes via GpSimd (SWDGE): keeps both Sync's and ACT's
                # in-order queues free for prefetch reads / gate math
                nc.gpsimd.dma_start(y[b, j * P:(j + 1) * P, :], acc[:])

    es.close()


def build_nc(s_sh):
    nc = bacc.Bacc("TRN2", target_bir_lowering=False, debug=False,
                   num_devices=N_CORES)
    x_ap = nc.dram_tensor("x", [D, B, s_sh, C], F32, kind="ExternalInput").ap()
    w_ap = nc.dram_tensor("w", [G, C], F32, kind="ExternalInput").ap()
    y_ap = nc.dram_tensor("y", [B, s_sh, C], F32, kind="ExternalOutput").ap()
    with tile.TileContext(nc) as tc:
        build_body(tc, x_ap, w_ap, y_ap, s_sh)
    nc.compile()
    return nc


_NC_CACHE = {}


def _get_nc(s_sh):
    if s_sh not in _NC_CACHE:
        _NC_CACHE[s_sh] = build_nc(s_sh)
    return _NC_CACHE[s_sh]


def run(cached_states, W_u, trace=False, trace_cores=None):
    s_sh = S // N_CORES
    nc = _get_nc(s_sh)
    xs = np.asarray(cached_states, dtype=np.float32)
    ws = np.ascontiguousarray(np.asarray(W_u, dtype=np.float32))
    in_maps = []
    for i in range(N_CORES):
        sh = np.ascontiguousarray(xs[:, :, i * s_sh:(i + 1) * s_sh, :])
        in_maps.append({"x": sh, "w": ws})
    res = bass_utils.run_bass_kernel_spmd(
        nc, in_maps, core_ids=list(range(N_CORES)), trace=trace,
        trace_cores=trace_cores)
    out = np.empty((B, S, C), np.float32)
    for i in range(N_CORES):
        out[:, i * s_sh:(i + 1) * s_sh, :] = res.results[i]["y"]
    return out, res


def kernel(cached_states, W_u):
    out, _ = run(cached_states, W_u)
    return out



# revision 6
# speedup vs baseline: 1.0821x; 1.0821x over previous
"""Trainium2 Bass kernel for nn_DepthMemoryCache.

Reference computation (D=8, B=4, S=4096, C=1024, G=64):
    u     = einsum('bsc,gc->bsg', x[-1], W_u)
    keys  = einsum('dbc,gc->dbg', x.mean(2), W_u)
    gates = softmax(einsum('bsg,dbg->bsd', u, keys), axis=-1)
    out   = einsum('dbsc,bsd->bsc', x, gates)

Strategy: shard the sequence axis over 8 cores (core i gets
x[:, :, i*512:(i+1)*512, :]).

Per core the kernel is a single continuous DMA stream with no idle gap:

  Phase A (batch-major): for each b, stream the 8 depth slabs (d=7 first).
  Depths 5..7 are cast to bf16 into a 12MB resident SBUF buffer (so phase B
  never re-reads them); all slabs are j-reduced on DVE/GpSimd (4 seq planes
  summed) so the per-(d,b) column sum needs only TWO N=512 PE matmuls with a
  one-hot indicator stationary into a per-b [D, C] PSUM region. uT = W_u@x7T
  blocks (PE transposes + matmuls off the resident bf16 x7) are interleaved
  so the tensor engine never spikes above ~50% and DMA runs at line rate.

  As soon as batch b's 8 slabs are summed, its [G, D] partial keys are
  fixed up (transpose + matmuls, ~2us) and AllReduduced on its own: 4 tiny
  2KB collectives, each fully hidden under the remaining phase-A streaming
  (a warm-up AllReduce at kernel start absorbs comm setup). By the time the
  last x slab lands, gates for b=0..2 are already computable, so phase B's
  streamed reads are issued back-to-back behind phase A's on the sync queue
  and the weighted-sum FMAs fire the moment the first tile lands.

  Phase B: per 128-row block, one small matmul for logits (uT was
  precomputed), softmax via ACT exp with accum_out, then 5 streamed fp32
  depth tiles + 3 resident bf16 depths combined by fused
  scalar_tensor_tensor FMAs with per-partition gate scalars on DVE.
  y writes ride the scalar engine's HWDGE ring so neither the sync read
  queue nor the gpsimd collective queue ever blocks.

HBM traffic per core: 64 (A) + 40 (B reads, 5/8 depths) + 8 (write)
= 112MB vs 136MB for the always-re-read variant. The bf16 mean/logit
paths and the 3 bf16-resident output terms cost ~1e-3 relative on a
2e-2 budget.
"""
import sys

sys.path.insert(0, "/opt/trn_rl_repo")

from contextlib import ExitStack

import numpy as np
from concourse import bacc, bass, mybir, tile, masks
from concourse import bass_utils

F32 = mybir.dt.float32
BF16 = mybir.dt.bfloat16

D, B, S, C, G = 8, 4, 4096, 1024, 64
N_CORES = 8
P = 128                 # partition count / block rows
NKC = C // P            # 8 column chunks of 128
NRES = 3                # depths D-NRES..D-1 stay resident in SBUF as bf16
RES0 = D - NRES


def build_body(tc, x, w, y, s_sh):
    """Emit the kernel IR. x:[D,B,s_sh,C], w:[G,C], y:[B,s_sh,C] dram APs."""
    nc = tc.nc
    nj = s_sh // P      # 128-row blocks per (d, b)
    mul, add = mybir.AluOpType.mult, mybir.AluOpType.add
    es = ExitStack()

    singles = es.enter_context(tc.tile_pool(name="singles", bufs=1))
    ident = singles.tile([P, P], F32)
    masks.make_identity(nc, ident[:])
    ident_bf = singles.tile([P, P], BF16)
    masks.make_identity(nc, ident_bf[:])
    # indicator stationaries: ind[:, d, m] = (m == d) / S — column-sums a
    # j-reduced bf16 plane into psum row d with one N=512 matmul per c-half.
    ind_bf = singles.tile([P, D, D], BF16)
    nc.vector.memset(ind_bf[:], 0.0)
    for d in range(D):
        nc.vector.memset(ind_bf[:, d, d:d + 1], 1.0 / (N_CORES * s_sh))
    w_sb = singles.tile([G, C], F32)
    nc.sync.dma_start(w_sb[:], w[:])
    # resident bf16 depths: xres[:, r] holds depth RES0+r; r=NRES-1 is d=D-1
    xres = singles.tile([P, NRES, B, nj, C], BF16)
    gates_sb = singles.tile([P, B, nj, D], F32)
    meanT_sb = singles.tile([P, NKC * D], F32)
    wT_sb = singles.tile([P, NKC, G], F32)
    wT_bf = singles.tile([P, NKC, G], BF16)
    keysT_sb = singles.tile([G, B, D], F32)
    uT_sb = singles.tile([G, B, nj, P], F32)

    bstream = es.enter_context(tc.tile_pool(name="bstream", bufs=7))

    dram = es.enter_context(tc.tile_pool(name="dram", bufs=1, space="DRAM"))
    # tiny warm-up AllReduce: absorbs collective-comm setup under phase A
    ccw_in = dram.tile([1, 16], F32)
    ccw_out = dram.tile([1, 16], F32)
    cc_in = [dram.tile([G, D], F32, name=f"cc_in{b}") for b in range(B)]
    cc_out = [dram.tile([G, D], F32, name=f"cc_out{b}") for b in range(B)]
    warm_sb = singles.tile([1, 16], F32)
    nc.vector.memset(warm_sb[:], 0.0)
    nc.gpsimd.dma_start(ccw_in[:], warm_sb[:])
    nc.gpsimd.collective_compute(
        "AllReduce", add, replica_groups=[list(range(N_CORES))],
        ins=[ccw_in.opt()], outs=[ccw_out.opt()],
    )

    # ---------------- Phase A: batch-major streaming + per-b collective ----
    with tc.tile_pool(name="psumA", bufs=1, space="PSUM") as psA, \
         tc.tile_pool(name="psumT", bufs=1, space="PSUM") as psT, \
         tc.tile_pool(name="psumXA", bufs=2, space="PSUM") as psXA, \
         tc.tile_pool(name="psumU", bufs=2, space="PSUM") as psU, \
         tc.tile_pool(name="xtA", bufs=3) as xtA, \
         tc.tile_pool(name="stream", bufs=2) as stream, \
         tc.tile_pool(name="jred", bufs=2) as jred, \
         tc.tile_pool(name="jbf", bufs=3) as jbf, \
         tc.tile_pool(name="sumsp", bufs=2) as sumsp, \
         tc.tile_pool(name="ksump", bufs=2) as ksump:

        # one-time W_u transpose: wT[c, g] chunks (fp32 + bf16 copies)
        for k in range(NKC):
            tr = psT.tile([P, G], F32, tag="wtr")
            nc.tensor.transpose(tr[:], w_sb[:, k * P:(k + 1) * P], ident[:G, :G])
            nc.vector.tensor_copy(wT_sb[:, k, :], tr[:])
            nc.scalar.copy(wT_bf[:, k, :], tr[:])

        sums_ps = psA.tile([D, C], F32)

        def sum_plane(plane_bf, d, first, last):
            # psum rows m != d get +0; one start/stop per 512-col bank per b
            for h in range(2):
                nc.tensor.matmul(
                    sums_ps[:, h * 512:(h + 1) * 512],
                    ind_bf[:, d, :],
                    plane_bf[:, h * 512:(h + 1) * 512],
                    start=first, stop=last,
                )

        def u_block(b, j):
            # uT[g, s-block] = sum_k (wT_k).T @ x7T_k on PE (reads resident
            # x7 bf16, so this can run any time after the d=D-1 cast)
            u_ps = psU.tile([G, P], F32, tag="u")
            for k in range(NKC):
                xt_ps = psXA.tile([P, P], BF16, tag="xt_ps")
                nc.tensor.transpose(
                    xt_ps[:], xres[:, NRES - 1, b, j, k * P:(k + 1) * P],
                    ident_bf[:])
                xt_sb = xtA.tile([P, P], BF16, tag="xt_sb")
                if k % 2 == 0:
                    nc.scalar.copy(xt_sb[:], xt_ps[:])
                else:
                    nc.vector.tensor_copy(xt_sb[:], xt_ps[:])
                nc.tensor.matmul(
                    u_ps[:], wT_bf[:, k, :], xt_sb[:],
                    start=(k == 0), stop=(k == NKC - 1))
            nc.vector.tensor_copy(uT_sb[:, b, j, :], u_ps[:])

        # depth order: d=7 first (fills resident x7 for u_blocks), resident
        # depths 5,6 last (their casts land directly in xres)
        dorder = [D - 1] + list(range(RES0)) + list(range(RES0, D - 1))
        for b in range(B):
            for di, d in enumerate(dorder):
                slab = stream.tile([P, nj, C], F32, tag="slab")
                nc.sync.dma_start(
                    slab[:], x[d, b].rearrange("(j p) c -> p j c", p=P))
                if d >= RES0:
                    # resident: bf16 casts into xres, then bf16 j-reduce
                    r = d - RES0
                    nc.vector.tensor_copy(xres[:, r, b, 0, :], slab[:, 0, :])
                    nc.scalar.copy(xres[:, r, b, 1, :], slab[:, 1, :])
                    nc.vector.tensor_copy(xres[:, r, b, 2, :], slab[:, 2, :])
                    nc.scalar.copy(xres[:, r, b, 3, :], slab[:, 3, :])
                    t01 = jbf.tile([P, C], BF16, tag="jbf")
                    t23 = jbf.tile([P, C], BF16, tag="jbf")
                    tfin = jbf.tile([P, C], BF16, tag="jbf")
                    nc.gpsimd.tensor_tensor(
                        t01[:], xres[:, r, b, 0, :], xres[:, r, b, 1, :], add)
                    nc.vector.tensor_tensor(
                        t23[:], xres[:, r, b, 2, :], xres[:, r, b, 3, :], add)
                    nc.gpsimd.tensor_tensor(tfin[:], t01[:], t23[:], add)
                    plane = tfin
                else:
                    # transient: fp32 j-reduce then one bf16 cast
                    t01 = jred.tile([P, C], F32, tag="jred")
                    t23 = jred.tile([P, C], F32, tag="jred")
                    nc.vector.tensor_tensor(
                        t01[:], slab[:, 0, :], slab[:, 1, :], add)
                    nc.gpsimd.tensor_tensor(
                        t23[:], slab[:, 2, :], slab[:, 3, :], add)
                    nc.vector.tensor_tensor(t01[:], t01[:], t23[:], add)
                    plane = jbf.tile([P, C], BF16, tag="jbf")
                    nc.scalar.copy(plane[:], t01[:])
                sum_plane(plane[:], d, first=(di == 0), last=(di == D - 1))
                # interleave one uT block during slabs 2..5 of this batch
                if 2 <= di <= 1 + nj:
                    u_block(b, di - 2)

            # ---- per-b fixup: meanT transpose + partial keysT + AllReduce --
            sums_sb = sumsp.tile([D, C], F32, tag="sums")
            nc.vector.tensor_copy(sums_sb[:], sums_ps[:])
            mt_ps = psT.tile([P, NKC * D], F32, tag="fix")
            for k in range(NKC):
                nc.tensor.matmul(
                    mt_ps[:, k * D:(k + 1) * D],
                    sums_sb[:, k * P:(k + 1) * P], ident[:D, :D],
                    is_transpose=True, start=(k == 0), stop=(k == NKC - 1))
            nc.vector.tensor_copy(meanT_sb[:], mt_ps[:])
            keys_ps = psT.tile([P, NKC * D], F32, tag="fix")
            for k in range(NKC):
                nc.tensor.matmul(
                    keys_ps[:G, :D],
                    wT_sb[:, k, :],
                    meanT_sb[:, k * D:(k + 1) * D],
                    start=(k == 0), stop=(k == NKC - 1),
                )
            ksum_sb = ksump.tile([G, D], F32, tag="ksum")
            nc.vector.tensor_copy(ksum_sb[:], keys_ps[:G, :D])
            # bounce DMAs ride GpSimd's queue so the Sync engine keeps
            # streaming; this b's AllReduce hides under later streaming
            nc.gpsimd.dma_start(cc_in[b][:], ksum_sb[:])
            nc.gpsimd.collective_compute(
                "AllReduce", add,
                replica_groups=[list(range(N_CORES))],
                ins=[cc_in[b].opt()], outs=[cc_out[b].opt()],
            )

        # bounce-out DMAs only after ALL streaming work is queued: each one
        # parks the gpsimd queue on its collective's completion semaphore,
        # which must not stall phase-A j-reduce work; by now b=0..2's
        # collectives are long done and b=3's finishes under phase B's
        # first-block streaming.
        for b in range(B):
            nc.gpsimd.dma_start(keysT_sb[:, b, :], cc_out[b][:])

    # ---------------- Phase B: gates + depth-weighted sum -------------------
    with tc.tile_pool(name="psumL", bufs=4, space="PSUM") as psL, \
         tc.tile_pool(name="accp", bufs=3) as accp, \
         tc.tile_pool(name="small", bufs=8) as small:
        for b in range(B):
            # gates for all 4 blocks of this b right after its AllReduce
            for j in range(nj):
                lg_ps = psL.tile([P, D], F32, tag="lg")
                nc.tensor.matmul(lg_ps[:], uT_sb[:, b, j, :], keysT_sb[:, b, :])
                e_sb = small.tile([P, D], F32, tag="e")
                z_sb = small.tile([P, 1], F32, tag="z")
                rz_sb = small.tile([P, 1], F32, tag="rz")
                nc.scalar.activation(
                    e_sb[:], lg_ps[:], mybir.ActivationFunctionType.Exp,
                    accum_out=z_sb[:])
                nc.vector.reciprocal(rz_sb[:], z_sb[:])
                nc.scalar.mul(gates_sb[:, b, j, :], e_sb[:], rz_sb[:])
            for j in range(nj):
                acc = accp.tile([P, C], F32, tag="acc")
                for d in range(RES0):        # streamed fp32 depths
                    t = bstream.tile([P, C], F32, tag="bslab")
                    nc.sync.dma_start(t[:], x[d, b, j * P:(j + 1) * P, :])
                    if d == 0:
                        nc.vector.tensor_scalar_mul(
                            acc[:], t[:], gates_sb[:, b, j, d:d + 1])
                    else:
                        nc.vector.scalar_tensor_tensor(
                            out=acc[:], in0=t[:],
                            scalar=gates_sb[:, b, j, d:d + 1],
                            in1=acc[:], op0=mul, op1=add)
                for r in range(NRES):        # resident bf16 depths
                    d = RES0 + r
                    nc.vector.scalar_tensor_tensor(
                        out=acc[:], in0=xres[:, r, b, j, :],
                        scalar=gates_sb[:, b, j, d:d + 1],
                        in1=acc[:], op0=mul, op1=add)
                # y writes via the ACT HWDGE ring: sync keeps reading,
                # gpsimd keeps its collective queue clear
                nc.scalar.dma_start(y[b, j * P:(j + 1) * P, :], acc[:])

    es.close()


def build_nc(s_sh):
    nc = bacc.Bacc("TRN2", target_bir_lowering=False, debug=False,
                   num_devices=N_CORES)
    x_ap = nc.dram_tensor("x", [D, B, s_sh, C], F32, kind="ExternalInput").ap()
    w_ap = nc.dram_tensor("w", [G, C], F32, kind="ExternalInput").ap()
    y_ap = nc.dram_tensor("y", [B, s_sh, C], F32, kind="ExternalOutput").ap()
    with tile.TileContext(nc) as tc:
        build_body(tc, x_ap, w_ap, y_ap, s_sh)
    nc.compile()
    return nc


_NC_CACHE = {}


def _get_nc(s_sh):
    if s_sh not in _NC_CACHE:
        _NC_CACHE[s_sh] = build_nc(s_sh)
    return _NC_CACHE[s_sh]


def run(cached_states, W_u, trace=False, trace_cores=None):
    s_sh = S // N_CORES
    nc = _get_nc(s_sh)
    xs = np.asarray(cached_states, dtype=np.float32)
    ws = np.ascontiguousarray(np.asarray(W_u, dtype=np.float32))
    in_maps = []
    for i in range(N_CORES):
        sh = np.ascontiguousarray(xs[:, :, i * s_sh:(i + 1) * s_sh, :])
        in_maps.append({"x": sh, "w": ws})
    res = bass_utils.run_bass_kernel_spmd(
        nc, in_maps, core_ids=list(range(N_CORES)), trace=trace,
        trace_cores=trace_cores)
    out = np.empty((B, S, C), np.float32)
    for i in range(N_CORES):
        out[:, i * s_sh:(i + 1) * s_sh, :] = res.results[i]["y"]
    return out, res


def kernel(cached_states, W_u):
    out, _ = run(cached_states, W_u)
    return out


# revision 7
# speedup vs baseline: 1.1604x; 1.0724x over previous
"""Trainium2 Bass kernel for nn_DepthMemoryCache.

Reference computation (D=8, B=4, S=4096, C=1024, G=64):
    u     = einsum('bsc,gc->bsg', x[-1], W_u)
    keys  = einsum('dbc,gc->dbg', x.mean(2), W_u)
    gates = softmax(einsum('bsg,dbg->bsd', u, keys), axis=-1)
    out   = einsum('dbsc,bsd->bsc', x, gates)

Strategy: shard the sequence axis over 8 cores (core i gets
x[:, :, i*512:(i+1)*512, :]). Per core the kernel is one continuous DMA
stream — both phases are DMA-bound and every compute engine stays well
under the read bandwidth:

  Phase A (batch-major): for each b, stream the 8 depth slabs (d=7
  first). Depths 5..7 are cast (on ACT, the cheapest engine for pure
  copies) to fp16 into a 12MB resident SBUF buffer, so phase B never
  re-reads them. Each slab is 4->1 j-reduced by a small DVE add tree
  into one fp16 plane, so the per-(d,b) column sum needs only TWO N=512
  PE matmuls with a one-hot indicator stationary into a [D, C] PSUM
  region. uT = W_u @ x7T blocks (PE transposes + matmuls off the
  resident fp16 x7) are interleaved one per slab.

  When batch b's slabs finish, its [G, D] partial keys are fixed up
  (transpose + matmuls, ~2us) and AllReduced on its own: 4 tiny 2KB
  collectives, each hidden under the remaining phase-A streaming (a
  warm-up AllReduce absorbs comm setup). GpSimd carries ONLY the cc_in
  bounces + collective_compute ops (a collective parks its queue until
  the fabric completes, so nothing else may ride it); the cc_out
  bounces ride the scalar queue at the start of each phase-B batch,
  after their collective is long done.

  Phase B: streamed reads are issued back-to-back behind phase A's on
  the sync queue. Per 128-row block: one small matmul for logits,
  softmax via ACT exp with accum_out, then 5 streamed depth tiles
  (pre-cast fp32->fp16 on ACT) + 3 resident fp16 depths combined by a
  fully-16-bit scalar_tensor_tensor chain on DVE (packed 2x rate; the
  DVE is element-rate-bound, so fp16-in/fp16-acc halves its cost vs
  fp32). The last link emits the fp32 accumulator that is DMA'd out on
  the scalar HWDGE ring.

HBM traffic per core: 64 (A) + 40 (B reads, 5/8 depths) + 8 (write)
= 112MB. fp16 x-quantization + fp16 partial accumulation cost ~5e-4
relative on a 2e-2 budget.
"""
import sys

sys.path.insert(0, "/opt/trn_rl_repo")

from contextlib import ExitStack

import numpy as np
from concourse import bacc, bass, mybir, tile, masks
from concourse import bass_utils

F32 = mybir.dt.float32
F16 = mybir.dt.float16

D, B, S, C, G = 8, 4, 4096, 1024, 64
N_CORES = 8
P = 128                 # partition count / block rows
NKC = C // P            # 8 column chunks of 128
NRES = 3                # depths D-NRES..D-1 stay resident in SBUF as fp16
RES0 = D - NRES


def build_body(tc, x, w, y, s_sh):
    """Emit the kernel IR. x:[D,B,s_sh,C], w:[G,C], y:[B,s_sh,C] dram APs."""
    nc = tc.nc
    nj = s_sh // P      # 128-row blocks per (d, b)
    mul, add = mybir.AluOpType.mult, mybir.AluOpType.add
    es = ExitStack()

    singles = es.enter_context(tc.tile_pool(name="singles", bufs=1))
    ident = singles.tile([P, P], F32)
    masks.make_identity(nc, ident[:])
    ident_h = singles.tile([P, P], F16)
    masks.make_identity(nc, ident_h[:])
    # indicator stationaries: ind[:, d, m] = (m == d) / S — column-sums a
    # j-reduced fp16 plane into psum row d with one N=512 matmul per c-half.
    ind_h = singles.tile([P, D, D], F16)
    nc.vector.memset(ind_h[:], 0.0)
    for d in range(D):
        nc.vector.memset(ind_h[:, d, d:d + 1], 1.0 / (N_CORES * s_sh))
    w_sb = singles.tile([G, C], F32)
    nc.sync.dma_start(w_sb[:], w[:])
    # resident fp16 depths: xres[:, r] holds depth RES0+r; r=NRES-1 is d=D-1
    xres = singles.tile([P, NRES, B, nj, C], F16)
    gates_sb = singles.tile([P, B, nj, D], F32)
    meanT_sb = singles.tile([P, NKC * D], F32)
    wT_sb = singles.tile([P, NKC, G], F32)
    wT_h = singles.tile([P, NKC, G], F16)
    keysT_sb = singles.tile([G, B, D], F32)
    uT_sb = singles.tile([G, B, nj, P], F32)

    bstream = es.enter_context(tc.tile_pool(name="bstream", bufs=6))

    dram = es.enter_context(tc.tile_pool(name="dram", bufs=1, space="DRAM"))
    # tiny warm-up AllReduce: absorbs collective-comm setup under phase A
    ccw_in = dram.tile([1, 16], F32)
    ccw_out = dram.tile([1, 16], F32)
    cc_in = [dram.tile([G, D], F32, name=f"cc_in{b}") for b in range(B)]
    cc_out = [dram.tile([G, D], F32, name=f"cc_out{b}") for b in range(B)]
    warm_sb = singles.tile([1, 16], F32)
    nc.vector.memset(warm_sb[:], 0.0)
    nc.gpsimd.dma_start(ccw_in[:], warm_sb[:])
    nc.gpsimd.collective_compute(
        "AllReduce", add, replica_groups=[list(range(N_CORES))],
        ins=[ccw_in.opt()], outs=[ccw_out.opt()],
    )

    # ---------------- Phase A: batch-major streaming + per-b collective ----
    with tc.tile_pool(name="psumA", bufs=1, space="PSUM") as psA, \
         tc.tile_pool(name="psumT", bufs=1, space="PSUM") as psT, \
         tc.tile_pool(name="psumXA", bufs=2, space="PSUM") as psXA, \
         tc.tile_pool(name="psumU", bufs=2, space="PSUM") as psU, \
         tc.tile_pool(name="xtA", bufs=3) as xtA, \
         tc.tile_pool(name="stream", bufs=3) as stream, \
         tc.tile_pool(name="jtree", bufs=6) as jtree, \
         tc.tile_pool(name="sumsp", bufs=2) as sumsp, \
         tc.tile_pool(name="ksump", bufs=2) as ksump:

        # one-time W_u transpose: wT[c, g] chunks (fp32 + fp16 copies)
        for k in range(NKC):
            tr = psT.tile([P, G], F32, tag="wtr")
            nc.tensor.transpose(tr[:], w_sb[:, k * P:(k + 1) * P], ident[:G, :G])
            nc.vector.tensor_copy(wT_sb[:, k, :], tr[:])
            nc.scalar.copy(wT_h[:, k, :], tr[:])

        sums_ps = psA.tile([D, C], F32)

        def sum_plane(plane_h, d, first, last):
            # psum rows m != d get +0; one start/stop per 512-col bank per b
            for h in range(2):
                nc.tensor.matmul(
                    sums_ps[:, h * 512:(h + 1) * 512],
                    ind_h[:, d, :],
                    plane_h[:, h * 512:(h + 1) * 512],
                    start=first, stop=last,
                )

        def u_block(b, j):
            # uT[g, s-block] = sum_k (wT_k).T @ x7T_k on PE (reads resident
            # x7 fp16, so this can run any time after the d=D-1 cast)
            u_ps = psU.tile([G, P], F32, tag="u")
            for k in range(NKC):
                xt_ps = psXA.tile([P, P], F16, tag="xt_ps")
                nc.tensor.transpose(
                    xt_ps[:], xres[:, NRES - 1, b, j, k * P:(k + 1) * P],
                    ident_h[:])
                xt_sb = xtA.tile([P, P], F16, tag="xt_sb")
                if k % 2 == 0:
                    nc.scalar.copy(xt_sb[:], xt_ps[:])
                else:
                    nc.vector.tensor_copy(xt_sb[:], xt_ps[:])
                nc.tensor.matmul(
                    u_ps[:], wT_h[:, k, :], xt_sb[:],
                    start=(k == 0), stop=(k == NKC - 1))
            nc.vector.tensor_copy(uT_sb[:, b, j, :], u_ps[:])

        # depth order: d=7 first (fills resident x7 for u_blocks), resident
        # depths 5,6 last (their casts land directly in xres)
        dorder = [D - 1] + list(range(RES0)) + list(range(RES0, D - 1))
        for b in range(B):
            for di, d in enumerate(dorder):
                slab = stream.tile([P, nj, C], F32, tag="slab")
                nc.sync.dma_start(
                    slab[:], x[d, b].rearrange("(j p) c -> p j c", p=P))
                t01 = jtree.tile([P, C], F16, tag="jt")
                t23 = jtree.tile([P, C], F16, tag="jt")
                tfin = jtree.tile([P, C], F16, tag="jt")
                if d >= RES0:
                    # resident: fp16 casts into xres on ACT, tree reads fp16
                    r = d - RES0
                    for j in range(nj):
                        nc.scalar.copy(xres[:, r, b, j, :], slab[:, j, :])
                    nc.vector.tensor_tensor(
                        t01[:], xres[:, r, b, 0, :], xres[:, r, b, 1, :], add)
                    nc.vector.tensor_tensor(
                        t23[:], xres[:, r, b, 2, :], xres[:, r, b, 3, :], add)
                else:
                    # transient: fp32 adds with fp16 outputs, no extra cast
                    nc.vector.tensor_tensor(
                        t01[:], slab[:, 0, :], slab[:, 1, :], add)
                    nc.vector.tensor_tensor(
                        t23[:], slab[:, 2, :], slab[:, 3, :], add)
                nc.vector.tensor_tensor(tfin[:], t01[:], t23[:], add)
                sum_plane(tfin[:], d, first=(di == 0), last=(di == D - 1))
                # interleave one uT block during slabs 2..5 of this batch
                if 2 <= di <= 1 + nj:
                    u_block(b, di - 2)

            # ---- per-b fixup: meanT transpose + partial keysT + AllReduce --
            sums_sb = sumsp.tile([D, C], F32, tag="sums")
            nc.vector.tensor_copy(sums_sb[:], sums_ps[:])
            mt_ps = psT.tile([P, NKC * D], F32, tag="fix")
            for k in range(NKC):
                nc.tensor.matmul(
                    mt_ps[:, k * D:(k + 1) * D],
                    sums_sb[:, k * P:(k + 1) * P], ident[:D, :D],
                    is_transpose=True, start=(k == 0), stop=(k == NKC - 1))
            nc.vector.tensor_copy(meanT_sb[:], mt_ps[:])
            keys_ps = psT.tile([P, NKC * D], F32, tag="fix")
            for k in range(NKC):
                nc.tensor.matmul(
                    keys_ps[:G, :D],
                    wT_sb[:, k, :],
                    meanT_sb[:, k * D:(k + 1) * D],
                    start=(k == 0), stop=(k == NKC - 1),
                )
            ksum_sb = ksump.tile([G, D], F32, tag="ksum")
            nc.vector.tensor_copy(ksum_sb[:], keys_ps[:G, :D])
            # gpsimd carries ONLY the collective chain: a collective_compute
            # parks its queue until the fabric completes, so no streaming
            # work may ride behind it
            nc.gpsimd.dma_start(cc_in[b][:], ksum_sb[:])
            nc.gpsimd.collective_compute(
                "AllReduce", add,
                replica_groups=[list(range(N_CORES))],
                ins=[cc_in[b].opt()], outs=[cc_out[b].opt()],
            )

    # ---------------- Phase B: gates + depth-weighted sum -------------------
    with tc.tile_pool(name="psumL", bufs=4, space="PSUM") as psL, \
         tc.tile_pool(name="acc16", bufs=3) as acc16p, \
         tc.tile_pool(name="accf", bufs=3) as accfp, \
         tc.tile_pool(name="bcast", bufs=6) as bcastp, \
         tc.tile_pool(name="small", bufs=8) as small:
        for b in range(B):
            # cc_out bounce on the scalar queue: collective b completed under
            # phase-A streaming (b=3's under phase B's first blocks)
            nc.scalar.dma_start(keysT_sb[:, b, :], cc_out[b][:])
            # gates for all 4 blocks of this b
            for j in range(nj):
                lg_ps = psL.tile([P, D], F32, tag="lg")
                nc.tensor.matmul(lg_ps[:], uT_sb[:, b, j, :], keysT_sb[:, b, :])
                e_sb = small.tile([P, D], F32, tag="e")
                z_sb = small.tile([P, 1], F32, tag="z")
                rz_sb = small.tile([P, 1], F32, tag="rz")
                nc.scalar.activation(
                    e_sb[:], lg_ps[:], mybir.ActivationFunctionType.Exp,
                    accum_out=z_sb[:])
                nc.vector.reciprocal(rz_sb[:], z_sb[:])
                nc.scalar.mul(gates_sb[:, b, j, :], e_sb[:], rz_sb[:])
            for j in range(nj):
                acc16 = acc16p.tile([P, C], F16, tag="a16")
                accf = accfp.tile([P, C], F32, tag="af")
                # d=0: ACT fuses the g0 scale with the fp32->fp16 downcast
                t0 = bstream.tile([P, C], F32, tag="bslab")
                nc.sync.dma_start(t0[:], x[0, b, j * P:(j + 1) * P, :])
                nc.scalar.activation(
                    acc16[:], t0[:], mybir.ActivationFunctionType.Copy,
                    scale=gates_sb[:, b, j, 0:1])
                for d in range(1, RES0):     # streamed fp32 depths
                    t = bstream.tile([P, C], F32, tag="bslab")
                    nc.sync.dma_start(t[:], x[d, b, j * P:(j + 1) * P, :])
                    th = bcastp.tile([P, C], F16, tag="bc")
                    nc.scalar.copy(th[:], t[:])
                    nc.vector.scalar_tensor_tensor(
                        out=acc16[:], in0=th[:],
                        scalar=gates_sb[:, b, j, d:d + 1],
                        in1=acc16[:], op0=mul, op1=add)
                for r in range(NRES):        # resident fp16 depths
                    d = RES0 + r
                    last = (r == NRES - 1)
                    nc.vector.scalar_tensor_tensor(
                        out=(accf[:] if last else acc16[:]),
                        in0=xres[:, r, b, j, :],
                        scalar=gates_sb[:, b, j, d:d + 1],
                        in1=acc16[:], op0=mul, op1=add)
                # y writes via the ACT HWDGE ring: sync keeps reading,
                # gpsimd keeps its collective queue clear
                nc.scalar.dma_start(y[b, j * P:(j + 1) * P, :], accf[:])

    es.close()


def build_nc(s_sh):
    nc = bacc.Bacc("TRN2", target_bir_lowering=False, debug=False,
                   num_devices=N_CORES)
    x_ap = nc.dram_tensor("x", [D, B, s_sh, C], F32, kind="ExternalInput").ap()
    w_ap = nc.dram_tensor("w", [G, C], F32, kind="ExternalInput").ap()
    y_ap = nc.dram_tensor("y", [B, s_sh, C], F32, kind="ExternalOutput").ap()
    with tile.TileContext(nc) as tc:
        build_body(tc, x_ap, w_ap, y_ap, s_sh)
    nc.compile()
    return nc


_NC_CACHE = {}


def _get_nc(s_sh):
    if s_sh not in _NC_CACHE:
        _NC_CACHE[s_sh] = build_nc(s_sh)
    return _NC_CACHE[s_sh]


def run(cached_states, W_u, trace=False, trace_cores=None):
    s_sh = S // N_CORES
    nc = _get_nc(s_sh)
    xs = np.asarray(cached_states, dtype=np.float32)
    ws = np.ascontiguousarray(np.asarray(W_u, dtype=np.float32))
    in_maps = []
    for i in range(N_CORES):
        sh = np.ascontiguousarray(xs[:, :, i * s_sh:(i + 1) * s_sh, :])
        in_maps.append({"x": sh, "w": ws})
    res = bass_utils.run_bass_kernel_spmd(
        nc, in_maps, core_ids=list(range(N_CORES)), trace=trace,
        trace_cores=trace_cores)
    out = np.empty((B, S, C), np.float32)
    for i in range(N_CORES):
        out[:, i * s_sh:(i + 1) * s_sh, :] = res.results[i]["y"]
    return out, res


def kernel(cached_states, W_u):
    out, _ = run(cached_states, W_u)
    return out
